# revision 26
# baseline (speedup 1.0000x reference)
import sys
from contextlib import ExitStack

import numpy as np

sys.path.insert(0, "/opt/trn_rl_repo")

import concourse.bass as bass  # noqa: E402
import concourse.mybir as mybir  # noqa: E402
import concourse.tile as tile  # noqa: E402
from concourse import bacc  # noqa: E402
from concourse.bass_utils import run_bass_kernel_spmd  # noqa: E402

C = 64
N_CORES = 8

F16 = mybir.dt.float16
F32 = mybir.dt.float32

# Half-image layout: partitions 0:64 = channels of image rows 0..63,
# partitions 64:128 = channels of rows 64..127.  A single padded fp16 plane
# [128, (H/2+2)*(W+2)] serves every tap as a plain offset view (identical
# offset for both halves), so no pre-shifted staging copies are needed and
# the output needs no cross-partition fold.
#
# Per tap k=(di,dj): f_k = blockdiag(Wk^T, Wk^T) @ x  (K=128 matmul, both
# halves at once), then T_k = (f_k + b_k) * patch_k elementwise, and
# out = sum_k T_k.
#
# Product routing (per 9 taps):
#   EXT  (dj=0 trio + dj=2 pair): ScalarE extracts f+b to fp16, DVE
#        multiplies 3/2 taps per op via 4D strided window APs (2x fp16 rate)
#   SDVE (tap 8): DVE scalar_tensor_tensor straight from PSUM
#   POOL (dj=1 trio): GpSimd scalar_tensor_tensor straight from PSUM
# Fold: DVE pairwise adds reduce the 5 extracted products to one tile;
# PE identity matmuls accumulate that tile + the 4 PSUM-route tiles into
# the output psum, which ScalarE copies out for the store DMA.

EXT = {0: 0, 3: 1, 6: 2, 2: 3, 5: 4, 8: 5}  # tap -> FB slot; dj=0 and dj=2 trios
SDVE = [1, 4, 7]  # dj=1 trio: DVE STT straight from PSUM (GpSimd can't read PSUM)


def pack_weights(w_gen: np.ndarray, b_gen: np.ndarray):
    W3 = w_gen.reshape(C, 9, C).astype(np.float32)  # [c, k, c']
    b3 = b_gen.reshape(C, 9).astype(np.float32)
    wt = np.zeros((128, 9 * 128), np.float32)
    bias = np.zeros((128, 9), np.float32)
    for k in range(9):
        blk = W3[:, k, :].T  # [c', c]
        wt[0:C, k * 128 : k * 128 + C] = blk
        wt[C:128, k * 128 + C : k * 128 + 128] = blk
        bias[0:C, k] = b3[:, k]
        bias[C:128, k] = b3[:, k]
    idt = np.eye(128, dtype=np.float32)
    return wt.astype(np.float16), bias.astype(np.float32), idt.astype(np.float16)


def build_nc(H=128, W=128, CH=8):
    HH = H // 2  # rows per half
    S = W  # packed row stride: no pad columns (fast contiguous input DMA)
    PR = HH + 2  # rows per half-plane incl halo rows
    PS = PR * S
    Nc = CH * W  # pixels per chunk (per half)
    nch = HH // CH
    mm_cols = 512  # psum-bank limit for fp32 matmul output
    rpm = mm_cols // W
    nmm = Nc // mm_cols

    nc = bacc.Bacc("TRN2", target_bir_lowering=False)
    x_in = nc.declare_dram_parameter("x", [C, H, W], F32, isOutput=False)
    wt_in = nc.declare_dram_parameter("wt", [128, 9 * 128], F16, isOutput=False)
    bias_in = nc.declare_dram_parameter("bias", [128, 9], F32, isOutput=False)
    idt_in = nc.declare_dram_parameter("idt", [128, 128], F16, isOutput=False)
    out_ext = nc.declare_dram_parameter("out", [C, H, W], F32, isOutput=True)

    add = mybir.AluOpType.add
    mult = mybir.AluOpType.mult
    Identity = mybir.ActivationFunctionType.Identity

    with ExitStack() as ctx:
        tc = ctx.enter_context(tile.TileContext(nc))
        const = ctx.enter_context(tc.tile_pool(name="const", bufs=1))
        fpsum = ctx.enter_context(tc.tile_pool(name="fpsum", bufs=3, space="PSUM"))
        opsum = ctx.enter_context(tc.tile_pool(name="opsum", bufs=1, space="PSUM"))
        fbp = ctx.enter_context(tc.tile_pool(name="fbp", bufs=3))
        ptp = ctx.enter_context(tc.tile_pool(name="ptp", bufs=3))
        pp = ctx.enter_context(tc.tile_pool(name="pp", bufs=9))
        dp = ctx.enter_context(tc.tile_pool(name="dp", bufs=6))
        outp = ctx.enter_context(tc.tile_pool(name="outp", bufs=3))

        # 1-elem head + S-elem tail margin: flat product windows read one
        # element past each end at the image borders (masked via FB zeroing)
        X = const.tile([128, 1 + PS + S], F16)
        WT = const.tile([128, 9 * 128], F16)
        BIAS = const.tile([128, 9], F32)
        IDT = const.tile([128, 128], F16)

        SCR = const.tile([128, 1], F32)
        nc.scalar.activation(SCR[:], SCR[:], Identity)
        nc.sync.dma_start(WT[:, 0:256], wt_in[:, 0:256])
        nc.sync.dma_start(BIAS[:], bias_in[:])
        nc.sync.dma_start(WT[:, 256:], wt_in[:, 256:])
        nc.sync.dma_start(IDT[:], idt_in[:])

        x3 = X[:, 1 : 1 + PS].rearrange("p (h w) -> p h w", h=PR)

        # zero the halo rows (top halo of the top half, bottom halo of the
        # bottom half).  No pad columns: edge-column wrap reads are masked by
        # zeroing the FB edge columns instead.
        nc.vector.memset(x3[0:C, 0, :], 0.0)
        nc.vector.memset(x3[C:128, PR - 1, :], 0.0)
        # zero the head/tail margins: they are multiplied by zeroed FB edge
        # columns, but uninitialized SBUF can hold NaN patterns (0*NaN=NaN)
        nc.vector.memset(X[:, 0:1], 0.0)
        nc.vector.memset(X[:, 1 + PS : 1 + PS + S], 0.0)

        # banded cast-loads (fp32 -> fp16): fully contiguous on both sides
        # (one descriptor per partition per band).  Early bands are small so
        # chunk 0 starts quickly.
        bands = [0, 4, 8, 12] + [12 + CH * i for i in range(1, nch - 1)] + [PR - 1]

        def emit_band(b):
            r0, r1 = bands[b], bands[b + 1]
            nc.gpsimd.dma_start(
                out=x3[0:C, 1 + r0 : 1 + r1, :],
                in_=x_in[:, r0:r1, :],
            )
            nc.gpsimd.dma_start(
                out=x3[C:128, r0:r1, :],
                in_=x_in[:, HH - 1 + r0 : HH - 1 + r1, :],
            )

        for b in range(3):
            emit_band(b)

        def win_flat(base_row, dj, count):
            """[128, count, CH*S] trio window: per tap one contiguous stretch
            over the packed plane; the horizontal shift is a +-1 element
            offset (row-wrap at edge columns is masked by FB zeroing)."""
            off = 1 + base_row * S + (dj - 1)
            base = X[:, off : off + 1]
            w = base.copy()
            w.ap = mybir.VecI64Pair([tuple(w.ap[0]), (S, count), (1, CH * S)])
            return w

        prev_fold = None  # (rhs AP lists, r0) from the previous chunk
        for n in range(nch):
            for b in range(n + 3, min(n + 4, len(bands) - 1)):
                emit_band(b)
            r0 = n * CH
            FB = fbp.tile([128, 6 * Nc], F16, tag="fb")
            FB3 = FB[:].rearrange("p (e h w) -> p e h w", e=6, w=W)
            FBf = FB[:].rearrange("p (e q) -> p e q", q=Nc)
            Tp = [None] * 9
            for k in [0, 1, 3, 4, 6, 7, 2, 8, 5]:
                di, dj = k // 3, k % 3
                fp = fpsum.tile([128, Nc], F32, tag="fp")
                for m in range(nmm):
                    rr = r0 + 1 + m * rpm
                    nc.tensor.matmul(
                        fp[:, m * mm_cols : (m + 1) * mm_cols],
                        WT[:, k * 128 : (k + 1) * 128],
                        x3[:, rr : rr + rpm, :],
                        start=True,
                        stop=True,
                    )
                if k in EXT:
                    nc.scalar.activation(
                        FBf[:, EXT[k]],
                        fp[:],
                        Identity,
                        bias=BIAS[:, k : k + 1],
                    )
                else:
                    P = pp.tile([128, Nc], F16, tag=f"p{k}")
                    nc.vector.scalar_tensor_tensor(
                        P[:], fp[:], BIAS[:, k : k + 1],
                        X[:, 1 + (r0 + di) * S : 1 + (r0 + di) * S + Nc],
                        add, mult,
                    )
                    Tp[k] = P

            # mask the edge-column row-wrap: the dj=0 taps' products must be
            # zero at image column 0, the dj=2 taps' at column W-1
            nc.gpsimd.memset(FB3[:, 0:3, :, 0], 0.0)
            nc.gpsimd.memset(FB3[:, 3:6, :, W - 1], 0.0)

            # products for the extracted taps: one flat DVE op per tap trio
            PT = ptp.tile([128, 6 * Nc], F16, tag="pt")
            PTf = PT[:].rearrange("p (e q) -> p e q", q=Nc)
            nc.vector.tensor_tensor(
                PTf[:, 0:3], FBf[:, 0:3], win_flat(r0, 0, 3), mult
            )
            nc.vector.tensor_tensor(
                PTf[:, 3:6], FBf[:, 3:6], win_flat(r0, 2, 3), mult
            )
            # GpSimd folds two of the STT product tiles into one
            D1 = dp.tile([128, Nc], F16, tag="d1")
            nc.gpsimd.tensor_tensor(D1[:], Tp[1][:], Tp[4][:], add)

            # PE identity-fold of the previous chunk (so the PE never waits
            # on this chunk's products): accumulate 8 tiles into out psum
            if prev_fold is not None:
                emit_fold(nc, tc, opsum, outp, out_ext, IDT, prev_fold, H, W, CH,
                          mm_cols)
            rhs = [
                [PT[:, i * Nc + m * mm_cols : i * Nc + (m + 1) * mm_cols]
                 for m in range(nmm)]
                for i in range(6)
            ] + [
                [t[:, m * mm_cols : (m + 1) * mm_cols] for m in range(nmm)]
                for t in (Tp[7], D1)
            ]
            prev_fold = (rhs, r0)

        emit_fold(nc, tc, opsum, outp, out_ext, IDT, prev_fold, H, W, CH, mm_cols,
                  split=True)

    nc.compile()
    return nc


def emit_fold(nc, tc, opsum, outp, out_ext, IDT, fold, H, W, CH, mm_cols,
              split=False):
    rhs, r0 = fold
    Nc = CH * W
    nmm = Nc // mm_cols
    op = opsum.tile([128, Nc], F32, tag="op")
    OUT = outp.tile([128, Nc], F32, tag="out")
    o3 = OUT[:].rearrange("p (a b) -> p a b", a=CH)
    hc = CH // nmm
    morder = range(nmm) if not split else range(nmm)
    if split:
        # last chunk: finish each 512-col half fully (fold, copy, store) so
        # the final store DMA starts as early as possible
        for m in range(nmm):
            sl = slice(m * mm_cols, (m + 1) * mm_cols)
            for fi, aps in enumerate(rhs):
                nc.tensor.matmul(
                    op[:, sl], IDT[:], aps[m],
                    start=(fi == 0), stop=(fi == len(rhs) - 1),
                )
            nc.scalar.copy(OUT[:, sl], op[:, sl])
            ra, rb = r0 + m * hc, r0 + (m + 1) * hc
            nc.sync.dma_start(out_ext[:, ra:rb, :], o3[0:64, m * hc : (m + 1) * hc])
            nc.sync.dma_start(out_ext[:, H // 2 + ra : H // 2 + rb, :],
                              o3[64:128, m * hc : (m + 1) * hc])
        return
    for fi, aps in enumerate(rhs):
        for m, ap in enumerate(aps):
            sl = slice(m * mm_cols, (m + 1) * mm_cols)
            nc.tensor.matmul(
                op[:, sl],
                IDT[:],
                ap,
                start=(fi == 0),
                stop=(fi == len(rhs) - 1),
            )
    nc.scalar.copy(OUT[:], op[:])
    nc.sync.dma_start(out_ext[:, r0 : r0 + CH, :], o3[0:64])
    nc.sync.dma_start(out_ext[:, H // 2 + r0 : H // 2 + r0 + CH, :], o3[64:128])


_NC_CACHE = {}


def _get_nc(H, W, CH):
    key = (H, W, CH)
    if key not in _NC_CACHE:
        _NC_CACHE[key] = build_nc(H, W, CH)
    return _NC_CACHE[key]


def run(x, w_gen, b_gen, trace=False, tmpdir=None):
    x = np.asarray(x, dtype=np.float32)
    w_gen = np.asarray(w_gen, dtype=np.float32)
    b_gen = np.asarray(b_gen, dtype=np.float32)
    B, c, H, W = x.shape
    assert c == C and B == N_CORES

    wt, bias, idt = pack_weights(w_gen, b_gen)
    nc = _get_nc(H, W, 8)

    in_maps = [
        {"x": np.ascontiguousarray(x[i]), "wt": wt, "bias": bias, "idt": idt}
        for i in range(B)
    ]
    res = run_bass_kernel_spmd(
        nc, in_maps, core_ids=list(range(N_CORES)), trace=trace, tmpdir=tmpdir
    )
    out = np.stack([res.results[i]["out"] for i in range(B)], axis=0)
    return out, res


def kernel(x: np.ndarray, w_gen: np.ndarray, b_gen: np.ndarray) -> np.ndarray:
    return run(x, w_gen, b_gen)[0]
